# revision 14
# baseline (speedup 1.0000x reference)
"""Column-wise RMS normalization on 8 Trainium2 NeuronCores.

Computes y = x * rsqrt(sum(x*x, axis=0) + eps) for x [32768, 2048] f32.

Sharding: column-parallel — each core owns a contiguous block of 256
columns, making the per-column sum-of-squares entirely core-local (no
collectives). Within a core the shard is viewed as [128 p, 256 t, 256 c]
(row = p*256 + t) so every DMA moves >=8KB contiguous runs per partition.

Single-read strategy: the f32 shard is DMA'd from HBM exactly once,
cast to fp16 on the fly (SWDGE cast DMA) into a persistent SBUF cache
(16MB/core). Pass A squares the cache (DVE) and reduces over partitions
with TensorE ones-matmuls into PSUM; the scale rsqrt(u+eps) is computed
via ACT Sqrt + DVE reciprocal and broadcast to all partitions with a
K=1 matmul. Pass B multiplies the cached fp16 x by the broadcast scale
(DVE) and DMAs f32 results out. HBM traffic = 32MB in + 32MB out per
core, the roofline floor.
"""

import numpy as np

import concourse.bacc as bacc
import concourse.bass as bass
import concourse.tile as tile
from concourse import mybir
from concourse.bass_utils import run_bass_kernel_spmd

N, D = 32768, 2048
EPS = 1e-6
NCORES = 8
C = D // NCORES  # 256 columns per core
P = 128          # partitions
T = N // P       # 256 rows per partition
G = 8            # row-group (t) per DMA / compute chunk
NG = T // G      # 32 groups

_NC = None


def _build() -> bass.Bass:
    nc = bacc.Bacc("TRN2", target_bir_lowering=False)
    x = nc.dram_tensor("x", [N, C], mybir.dt.float32, kind="ExternalInput")
    y = nc.dram_tensor("y", [N, C], mybir.dt.float32, kind="ExternalOutput")
    xv = x[:, :].rearrange("(p t) c -> p t c", p=P)
    yv = y[:, :].rearrange("(p t) c -> p t c", p=P)

    with tile.TileContext(nc) as tc:
        with (
            tc.tile_pool(name="cache", bufs=1) as cachep,
            tc.tile_pool(name="consts", bufs=1) as consts,
            tc.tile_pool(name="sq", bufs=2) as sqp,
            tc.tile_pool(name="outs", bufs=3) as outp,
            tc.tile_pool(name="scale", bufs=1) as scalep,
            tc.tile_pool(name="ps", bufs=1, space="PSUM") as psp,
        ):
            xc = cachep.tile([P, T, C], mybir.dt.float16)
            ones_col = consts.tile([P, 1], mybir.dt.float16)
            nc.vector.memset(ones_col, 1.0)
            ones_row = consts.tile([1, P], mybir.dt.float32)
            nc.vector.memset(ones_row, 1.0)
            eps_t = consts.tile([P, 1], mybir.dt.float32)
            nc.vector.memset(eps_t, EPS)

            # u_ps holds 2 interleaved partial column-sum vectors (even/odd t)
            u_ps = psp.tile([1, 2 * C], mybir.dt.float32)
            ubc_ps = psp.tile([P, C], mybir.dt.float32)

            # Pass A: cast-DMA f32->fp16 into the persistent cache (SWDGE),
            # square on DVE, reduce over partitions (PE ones-matmul
            # accumulate into PSUM). The final 8 rows are processed in
            # small G=2 chunks so the tail of the dependency chain into the
            # scale computation (last square + last matmul) is short.
            # 2MB cast-DMAs for the bulk, one 1MB group, then a short G=2
            # tail to keep the final square->matmul chain off the critical
            # path into the scale computation.
            GI = 2 * G
            in_groups = (
                [(j * GI, GI) for j in range(T // GI - 1)]
                + [(T - GI, G)]
                + [(T - G + 2 * h, 2) for h in range(G // 2)]
            )
            nmm = T // 2
            k = 0
            for t0, g in in_groups:
                ts_ = slice(t0, t0 + g)
                nc.gpsimd.dma_start(out=xc[:, ts_, :], in_=xv[:, ts_, :])
                sq = sqp.tile([P, g, C], mybir.dt.float16, tag="sq")
                nc.vector.tensor_mul(sq, xc[:, ts_, :], xc[:, ts_, :])
                for h in range(g // 2):
                    rhs = sq[:, 2 * h : 2 * h + 2, :].rearrange("p t c -> p (t c)")
                    nc.tensor.matmul(
                        u_ps[:, :],
                        lhsT=ones_col[:, :],
                        rhs=rhs,
                        start=(k == 0),
                        stop=(k == nmm - 1),
                    )
                    k += 1

            # Scale: u = even+odd partials; s = 1/sqrt(u+eps), broadcast to 128p
            u_sb = scalep.tile([1, C], mybir.dt.float32)
            upair = u_ps[:, :].rearrange("p (t c) -> p c t", t=2)
            nc.vector.reduce_sum(u_sb, upair, axis=mybir.AxisListType.X)
            nc.tensor.matmul(
                ubc_ps[:, :], lhsT=ones_row[:, :], rhs=u_sb[:, :], start=True, stop=True
            )
            tsq = scalep.tile([P, C], mybir.dt.float32)
            nc.scalar.activation(
                out=tsq[:, :],
                in_=ubc_ps[:, :],
                func=mybir.ActivationFunctionType.Sqrt,
                bias=eps_t[:, :],
                scale=1.0,
            )
            s_bc = scalep.tile([P, 1, C], mybir.dt.float32)
            nc.vector.reciprocal_approx_fast(out=s_bc[:, 0, :], in_=tsq[:, :])

            # Pass B: scale cached x, write out. First 8 rows go in small
            # G=2 chunks so the first out-DMA launches as early as possible
            # after the scale is ready.
            out_groups = [(2 * h, 2) for h in range(G // 2)] + [
                (G + j * G, G) for j in range(NG - 1)
            ]
            for t0, g in out_groups:
                ts_ = slice(t0, t0 + g)
                ot = outp.tile([P, g, C], mybir.dt.float32, tag="ot")
                nc.vector.tensor_mul(
                    ot, xc[:, ts_, :], s_bc[:, :, :].to_broadcast((P, g, C))
                )
                nc.sync.dma_start(out=yv[:, ts_, :], in_=ot)
    nc.compile()
    return nc


def _get_nc() -> bass.Bass:
    global _NC
    if _NC is None:
        _NC = _build()
    return _NC


def kernel(x) -> np.ndarray:
    x = np.asarray(x, dtype=np.float32)
    assert x.shape == (N, D), x.shape
    nc = _get_nc()
    in_maps = [
        {"x": np.ascontiguousarray(x[:, i * C : (i + 1) * C])} for i in range(NCORES)
    ]
    res = run_bass_kernel_spmd(nc, in_maps, core_ids=list(range(NCORES)))
    return np.concatenate([r["y"] for r in res.results], axis=1)
